# revision 11
# baseline (speedup 1.0000x reference)
"""Trainium2 Bass kernel for nn_DIAGCN (RGCN + GraphConv + classifier over
block-diagonal dialog graphs), SPMD over 8 NeuronCores.

Strategy
--------
The dialog graph is a causal 5-tap window (edges i -> i+o, o = 0..4, within
each 100-utterance dialog), and relation_type(i,j) = spk[i]*spk[j] with spk
derived from self-edges.  Every per-node linear map commutes with both the
window sum W(.) (row-mixing) and per-node diagonal scalings (row scaling), so
the whole network folds into 7-wide channels:

    out = W(g0) + f0
    g0  = RA + ic0.*W(A0) - ic0s.*W(spk.*A0) + ic1s.*W(spk.*A1) + cA*nv.*mask
    f0  = FSC + ic0.*W(B0) - ic0s.*W(spk.*B0) + ic1s.*W(spk.*B1) + cBc.*mask
    A{0,1} = x@(w_rel{0,1}@wA), B likewise with wB; RA = x@(w_root@wA),
    FSC = x@(w_root@wB + w_skip@w_clf); wA = w_gc_rel@w_clf, wB = w_gc_root@w_clf

Device work per column tile: one [1024 -> 56] bf16 matmul over x (the memory-
bound stream), one DVE multiply (ps * [spk|ones] -> Z rows 0:56), a 5-tap
window shift-tree over Z rows 0:42 batched two tiles per DVE op, a coefficient
multiply, and two tiny accumulating matmuls ([42->39] from the windowed V and
[32->39] from the plain Z rows + host-precomputed nv/mask constant rows).
The mask / neighbor-count channels are computed on host, so nothing else is
windowed on device.  Everything runs in bf16 (gate is 2e-2; this lands ~5e-3).

Layout: nodes sharded by dialog (no cross-core edges), 64 padded dialogs per
core; each dialog = 4 zero gap columns + 100 data columns so window sums never
leak across dialogs.  x ships transposed+tiled, one 1 MB DMA per column tile
(8 KiB contiguous per partition), all on the sync HWDGE ring (buffer-gated
issue); constants stream in parallel on the gpsimd SWDGE ring.
"""
import numpy as np
import ml_dtypes

BF16 = ml_dtypes.bfloat16

# ---------------------------------------------------------------- constants
B, L, FUT = 500, 100, 4
N = B * L
IN, HID, NCLS = 1024, 512, 7
NCORES = 8
GAP = 4
DLG = L + GAP            # 104 columns per dialog
DPC = 64                 # padded dialogs per core
COLS = DPC * DLG         # 6656 columns per core
NT = 13                  # column tiles
NTC = COLS // NT         # 512
KB = IN // 128           # 8 contraction blocks
M = 56                   # Wbig columns (psum partitions)
M2 = 39                  # S-matmul output columns
WIN = 42                 # windowed rows (0:28 spk-scaled, 28:42 plain)

D_COUNTS = [63, 63, 63, 63, 62, 62, 62, 62]
D_STARTS = np.concatenate([[0], np.cumsum(D_COUNTS)])[:-1]


def _data_cols():
    d = np.arange(DPC)[:, None]
    u = np.arange(L)[None, :]
    return d * DLG + GAP + u  # [DPC, L]


# ---------------------------------------------------------------- host prep
def _check_graph(edges, relation_type):
    i = np.arange(L)[:, None]
    off = np.arange(FUT + 1)[None, :]
    tl = i + off
    valid = tl < L
    sl = np.broadcast_to(i, tl.shape)[valid]
    tl = tl[valid]
    base = (np.arange(B) * L)[:, None]
    src = (base + sl[None, :]).reshape(-1)
    tgt = (base + tl[None, :]).reshape(-1)
    if edges.shape != (2, src.size) or not (
        np.array_equal(edges[0], src) and np.array_equal(edges[1], tgt)
    ):
        raise ValueError("edge structure does not match the DIAGCN pattern")
    sel = edges[0] == edges[1]
    spk = np.zeros(N, dtype=np.float64)
    spk[edges[0][sel]] = relation_type[sel]
    return spk


def _host_prep(x, edges, relation_type, w_rel, w_root, b_rgcn,
               w_gc_rel, w_gc_root, b_gc, w_skip, b_skip, w_clf, b_clf):
    x = np.asarray(x, dtype=np.float32)
    edges = np.asarray(edges)
    relation_type = np.asarray(relation_type)
    spk = _check_graph(edges, relation_type)

    tgt = edges[1]
    c1 = np.bincount(tgt[relation_type == 1], minlength=N).astype(np.float64)
    c0 = np.bincount(tgt[relation_type == 0], minlength=N).astype(np.float64)
    ic0 = 1.0 / np.maximum(c0, 1.0)
    ic1 = 1.0 / np.maximum(c1, 1.0)
    ic0s = ic0 * spk
    ic1s = ic1 * spk

    f8 = lambda a: np.asarray(a, dtype=np.float64)
    w_rel, w_root, w_gc_rel, w_gc_root, w_skip, w_clf = map(
        f8, (w_rel, w_root, w_gc_rel, w_gc_root, w_skip, w_clf))
    b_rgcn, b_gc, b_skip, b_clf = map(f8, (b_rgcn, b_gc, b_skip, b_clf))

    wA = w_gc_rel @ w_clf
    wB = w_gc_root @ w_clf
    # ps/Z rows: 0:7 A0(->A0S) 7:14 A1(->A1S) 14:21 B0(->B0S) 21:28 B1(->B1S)
    #            28:35 A0 plain, 35:42 B0 plain, 42:49 RA, 49:56 FSC
    Wbig = np.zeros((IN, M), dtype=np.float64)
    Wbig[:, 0:7] = w_rel[0] @ wA
    Wbig[:, 7:14] = w_rel[1] @ wA
    Wbig[:, 14:21] = w_rel[0] @ wB
    Wbig[:, 21:28] = w_rel[1] @ wB
    Wbig[:, 28:35] = w_rel[0] @ wA
    Wbig[:, 35:42] = w_rel[0] @ wB
    Wbig[:, 42:49] = w_root @ wA
    Wbig[:, 49:56] = w_root @ wB + w_skip @ w_clf
    # [128 partitions, KB, M]: partition p holds weight rows {k*128+p}
    Wbig = np.ascontiguousarray(
        Wbig.reshape(KB, 128, M).swapaxes(0, 1)).astype(BF16)

    cA = b_rgcn @ wA
    cBc = b_rgcn @ wB + (b_gc + b_skip) @ w_clf + b_clf
    # S_a: windowed+coefed V rows 0:42 -> ps2 (g0 at 0:7, f0 at 32:39)
    S_a = np.zeros((WIN, M2), dtype=np.float32)
    for i in range(7):
        S_a[0 + i, i] = 1.0          # -ic0s.*W(A0S)
        S_a[7 + i, i] = 1.0          # +ic1s.*W(A1S)
        S_a[28 + i, i] = 1.0         # ic0.*W(A0)
        S_a[14 + i, 32 + i] = 1.0    # -ic0s.*W(B0S)
        S_a[21 + i, 32 + i] = 1.0    # +ic1s.*W(B1S)
        S_a[35 + i, 32 + i] = 1.0    # ic0.*W(B0)
    # S_b: plain Z rows 32:64 -> ps2.  Z row 32+j maps to S_b row j:
    # rows 0:10 = A0[4:7],B0 plain (windowed copies, no direct contribution),
    # rows 10:17 = RA, 17:24 = FSC, 24 = nv*mask (cA), 25 = mask (cBc)
    S_b = np.zeros((32, M2), dtype=np.float32)
    for i in range(7):
        S_b[10 + i, i] = 1.0
        S_b[17 + i, 32 + i] = 1.0
    S_b[24, 0:7] = cA
    S_b[25, 32:39] = cBc
    S_a = S_a.astype(BF16)
    S_b = S_b.astype(BF16)

    dc = _data_cols()
    mask_col = np.zeros(COLS, dtype=np.float64)
    mask_col[dc.reshape(-1)] = 1.0
    nvm = np.convolve(mask_col, np.ones(FUT + 1))[:COLS] * mask_col
    zc = np.zeros((8, COLS), dtype=np.float32)  # -> Z rows 56:64
    zc[0] = nvm
    zc[1] = mask_col
    zc = zc.astype(BF16)

    in_maps = []
    unshard_info = []
    for c in range(NCORES):
        nd = D_COUNTS[c]
        g0 = D_STARTS[c]
        cols_real = dc[:nd].reshape(-1)
        nodes_real = g0 * L + np.arange(nd * L)

        xt = np.zeros((IN, COLS), dtype=np.float32)
        xt[:, cols_real] = x[nodes_real].T
        # swizzle: [NT][128 partitions][KB][NTC] so each column tile is one
        # DMA with 8 KiB contiguous per partition
        xts = np.ascontiguousarray(
            xt.reshape(KB, 128, NT, NTC).transpose(2, 1, 0, 3)).astype(BF16)

        def vec_to_cols(v):
            out = np.zeros(COLS, dtype=np.float32)
            out[cols_real] = v[nodes_real]
            return out

        spk_c = vec_to_cols(spk)
        ic0_c = vec_to_cols(ic0)
        ic0s_c = vec_to_cols(ic0s)
        ic1s_c = vec_to_cols(ic1s)

        spk32 = np.empty((32, COLS), dtype=np.float32)
        spk32[0:28] = spk_c
        spk32[28:32] = 1.0  # rows 28:32 of the spk|ones plane (rest is memset)
        coefrep = np.zeros((WIN, COLS), dtype=np.float32)
        coefrep[0:7] = -ic0s_c
        coefrep[7:14] = ic1s_c
        coefrep[14:21] = -ic0s_c
        coefrep[21:28] = ic1s_c
        coefrep[28:35] = ic0_c
        coefrep[35:42] = ic0_c

        in_maps.append(dict(
            xt=xts, wbig=Wbig, sa=S_a, sb=S_b, zc=zc,
            spk28=spk32.astype(BF16),
            coefrep=coefrep.astype(BF16),
        ))
        unshard_info.append((nodes_real, cols_real))
    return in_maps, unshard_info


# ---------------------------------------------------------------- bass kernel
_COMPILED = None


def _build():
    import concourse.bass as bass
    from concourse import bacc
    import concourse.mybir as mybir
    from concourse.tile import TileContext

    f32 = mybir.dt.float32
    bf16 = mybir.dt.bfloat16
    ADD = mybir.AluOpType.add
    MUL = mybir.AluOpType.mult

    nc = bacc.Bacc("TRN2", target_bir_lowering=False, debug=False,
                   num_devices=NCORES)
    xt_d = nc.dram_tensor("xt", [NT, 128, KB, NTC], bf16, kind="ExternalInput")
    wbig_d = nc.dram_tensor("wbig", [128, KB, M], bf16, kind="ExternalInput")
    sa_d = nc.dram_tensor("sa", [WIN, M2], bf16, kind="ExternalInput")
    sb_d = nc.dram_tensor("sb", [32, M2], bf16, kind="ExternalInput")
    zc_d = nc.dram_tensor("zc", [8, COLS], bf16, kind="ExternalInput")
    spk_d = nc.dram_tensor("spk28", [32, COLS], bf16, kind="ExternalInput")
    coef_d = nc.dram_tensor("coefrep", [WIN, COLS], bf16, kind="ExternalInput")
    y_d = nc.dram_tensor("y", [NCLS, COLS], bf16, kind="ExternalOutput")

    with TileContext(nc) as tc:
        with (
            tc.tile_pool(name="const", bufs=1) as cpool,
            tc.tile_pool(name="xin", bufs=2) as xpool,
            tc.tile_pool(name="wrk", bufs=3) as wpool,
            tc.tile_pool(name="g2", bufs=1) as gpool,
            tc.tile_pool(name="psum", bufs=4, space="PSUM") as ppool,
            tc.tile_pool(name="psum2", bufs=4, space="PSUM") as p2pool,
        ):
            wsb = cpool.tile([128, KB, M], bf16)
            nc.sync.dma_start(wsb[:], wbig_d[:])
            sasb = cpool.tile([WIN, M2], bf16)
            nc.sync.dma_start(sasb[:], sa_d[:])
            # S_b parks at partitions 32:64 — matmul requires lhsT and rhs to
            # share the same base partition, and its rhs is tZ[32:64]
            sbt = cpool.tile([64, M2], bf16)
            nc.sync.dma_start(sbt[32:64], sb_d[:])
            sbsb = sbt[32:64]

            # persistent [*, COLS] planes (SBUF free-dim bytes are charged per
            # partition regardless of row count).  Engine-op APs must start at
            # partition 0/32/64/96 and a non-zero base spans at most 32
            # partitions, so the >32-row tensors each sit at base 0.
            tZ = cpool.tile([64, COLS], bf16)    # Z: 0:56 per-tile, 56:64 consts
            tSP = cpool.tile([M, COLS], bf16)    # spk|ones
            tT1 = cpool.tile([WIN, COLS], bf16)  # window stage 1 (persists)
            tV = cpool.tile([WIN, COLS], bf16)   # windowed * coef
            tCF = cpool.tile([WIN, COLS], bf16)  # coefficients
            tGP = cpool.tile([128, COLS // 4], bf16)  # packed g0: group g rows 32g..32g+6
            tOP = cpool.tile([128, COLS // 4], bf16)  # packed out, same layout

            # concurrent DMAs share HBM bandwidth round-robin, so the first
            # tiles must be queued ahead of the (large) constant planes or the
            # first matmul starts late.  Everything rides the sync HWDGE ring;
            # xpool bufs=2 keeps the in-flight queue shallow for the same
            # reason.
            xts = {}
            for t in (0, 1):
                xts[t] = xpool.tile([128, KB, NTC], bf16, tag="xt", name="xt_t")
                nc.sync.dma_start(xts[t][:], xt_d[t])
            nc.sync.dma_start(tZ[56:64], zc_d[:])
            nc.sync.dma_start(tSP[0:32], spk_d[:])
            nc.sync.dma_start(tCF[:], coef_d[:])
            nc.vector.memset(tSP[32:M], 1.0)
            nc.vector.memset(tGP[:], 0.0)
            nc.vector.memset(tOP[:], 0.0)
            # dummy copy so the one-time ACT table load happens at startup,
            # not in front of the first real PSUM->SBUF copy
            nc.scalar.copy(tT1[0:1, 0:8], sbt[32:33, 0:8])

            GRP = COLS // 4  # 1664, a whole number of dialogs

            def finalize_pair(tlo, thi):
                # second S-matmul + PSUM->packed copies; called one pair late
                # so the tensor queue never stalls waiting on the DVE chain
                for t in range(tlo, thi):
                    c0, c1 = t * NTC, (t + 1) * NTC
                    ps2 = ps2s.pop(t)
                    nc.tensor.matmul(ps2[:], sasb[:], tV[:, c0:c1],
                                     start=False, stop=True)
                    for (glo, ghi) in [(c0, min(c1, (c0 // GRP + 1) * GRP)),
                                       ((c0 // GRP + 1) * GRP, c1)]:
                        if glo >= ghi:
                            continue
                        g = glo // GRP
                        nc.scalar.copy(
                            tGP[32 * g:32 * g + NCLS, glo - g * GRP:ghi - g * GRP],
                            ps2[0:NCLS, glo - c0:ghi - c0])
                        nc.scalar.copy(
                            tOP[32 * g:32 * g + NCLS, glo - g * GRP:ghi - g * GRP],
                            ps2[32:32 + NCLS, glo - c0:ghi - c0])

            # win2: 5-tap window of packed g0, all 4 groups per op (rows
            # 32g..32g+6).  Chunked so most of it overlaps the main loop.
            NR = 96 + NCLS
            gs1 = gpool.tile([NR, GRP], bf16, tag="gs1")
            gp = tGP[0:NR]

            def win2_chunk(b0, b1):
                gs2 = gpool.tile([NR, b1 - b0], bf16, tag="gs2")
                gwt = gpool.tile([NR, b1 - b0], bf16, tag="gwt")
                if b0 == 0:
                    nc.vector.tensor_copy(gs1[:, 0:1], gp[:, 0:1])
                    nc.vector.tensor_tensor(gs1[:, 1:b1], gp[:, 1:b1], gp[:, 0:b1 - 1], ADD)
                    nc.vector.tensor_copy(gs2[:, 0:2], gs1[:, 0:2])
                    nc.vector.tensor_tensor(gs2[:, 2:], gs1[:, 2:b1], gs1[:, 0:b1 - 2], ADD)
                    nc.vector.tensor_copy(gwt[:, 0:4], gs2[:, 0:4])
                    nc.vector.tensor_tensor(gwt[:, 4:], gs2[:, 4:], gp[:, 0:b1 - 4], ADD)
                else:
                    nc.vector.tensor_tensor(gs1[:, b0:b1], gp[:, b0:b1], gp[:, b0 - 1:b1 - 1], ADD)
                    nc.vector.tensor_tensor(gs2[:], gs1[:, b0:b1], gs1[:, b0 - 2:b1 - 2], ADD)
                    nc.vector.tensor_tensor(gwt[:], gs2[:], gp[:, b0 - 4:b1 - 4], ADD)
                nc.vector.tensor_tensor(tOP[0:NR, b0:b1], tOP[0:NR, b0:b1], gwt[:], ADD)

            B1 = 12 * NTC - 3 * GRP  # group-3 columns complete after tile 11

            ps2s = {}
            pairs = [(2 * p, min(2 * p + 2, NT)) for p in range((NT + 1) // 2)]
            for (tlo, thi) in pairs:
                for t in range(tlo, thi):
                    if t not in xts:
                        xts[t] = xpool.tile([128, KB, NTC], bf16, tag="xt", name="xt_t")
                        nc.sync.dma_start(xts[t][:], xt_d[t])
                    xt_t = xts.pop(t)
                    ps = ppool.tile([M, NTC], f32)
                    for k in range(KB):
                        nc.tensor.matmul(
                            ps[:], wsb[:, k, :], xt_t[:, k, :],
                            start=(k == 0), stop=(k == KB - 1))
                    c0, c1 = t * NTC, (t + 1) * NTC
                    # rows 0:28 spk-scaled, 28:56 plain (spkx rows are 1.0)
                    nc.vector.tensor_tensor(tZ[0:M, c0:c1], ps[:], tSP[:, c0:c1], MUL)
                    # first S-matmul half: plain rows + host consts, needs only Z
                    ps2s[t] = p2pool.tile([M2, NTC], f32, tag="ps2", name="ps2")
                    nc.tensor.matmul(ps2s[t][:], sbsb, tZ[32:64, c0:c1],
                                     start=True, stop=False)

                # 5-tap causal window over the pair's columns as a shift tree:
                #   t1 = z + sh1(z); t2 = t1 + sh2(t1); wt = t2 + sh4(z)
                C0, C1 = tlo * NTC, thi * NTC
                W = C1 - C0
                Z = tZ[0:WIN]
                T1 = tT1
                T2 = wpool.tile([WIN, W], bf16, tag="T2")
                WT = wpool.tile([WIN, W], bf16, tag="WT")
                if tlo == 0:
                    nc.vector.tensor_copy(T1[:, 0:1], Z[:, 0:1])
                    nc.vector.tensor_tensor(T1[:, 1:C1], Z[:, 1:C1], Z[:, 0:C1 - 1], ADD)
                    nc.vector.tensor_copy(T2[:, 0:2], T1[:, 0:2])
                    nc.vector.tensor_tensor(T2[:, 2:], T1[:, 2:C1], T1[:, 0:C1 - 2], ADD)
                    nc.vector.tensor_copy(WT[:, 0:4], T2[:, 0:4])
                    nc.vector.tensor_tensor(WT[:, 4:], T2[:, 4:], Z[:, 0:C1 - 4], ADD)
                else:
                    nc.vector.tensor_tensor(T1[:, C0:C1], Z[:, C0:C1], Z[:, C0 - 1:C1 - 1], ADD)
                    nc.vector.tensor_tensor(T2[:], T1[:, C0:C1], T1[:, C0 - 2:C1 - 2], ADD)
                    nc.vector.tensor_tensor(WT[:], T2[:], Z[:, C0 - 4:C1 - 4], ADD)
                nc.vector.tensor_tensor(tV[:, C0:C1], WT[:], tCF[:, C0:C1], MUL)

                # lag finalize by one pair (immediate for the last two pairs:
                # pair 5 so win2 chunk 1 can start early, pair 6 as the tail)
                prev = pairs.index((tlo, thi)) - 1
                if prev >= 0 and ps2s.get(pairs[prev][0]) is not None:
                    finalize_pair(*pairs[prev])
                if (tlo, thi) == pairs[-2]:
                    finalize_pair(tlo, thi)
                elif (tlo, thi) == pairs[-1]:
                    win2_chunk(0, B1)
                    finalize_pair(tlo, thi)

            win2_chunk(B1, GRP)
            for g in range(4):
                eng = nc.sync if g % 2 == 0 else nc.scalar
                eng.dma_start(y_d[:, g * GRP:(g + 1) * GRP],
                              tOP[32 * g:32 * g + NCLS, :])
    nc.compile()
    return nc


def _get_compiled():
    global _COMPILED
    if _COMPILED is None:
        _COMPILED = _build()
    return _COMPILED


def _run(in_maps, trace=False):
    from concourse.bass_utils import run_bass_kernel_spmd
    nc = _get_compiled()
    return run_bass_kernel_spmd(nc, in_maps, list(range(NCORES)), trace=trace)


def kernel(**inputs) -> np.ndarray:
    in_maps, unshard_info = _host_prep(**inputs)
    res = _run(in_maps)
    out = np.zeros((N, NCLS), dtype=np.float32)
    for c in range(NCORES):
        nodes_real, cols_real = unshard_info[c]
        out[nodes_real] = res.results[c]["y"][:, cols_real].T.astype(np.float32)
    return out


# revision 13
# speedup vs baseline: 1.0616x; 1.0616x over previous
"""Trainium2 Bass kernel for nn_DIAGCN (RGCN + GraphConv + classifier over
block-diagonal dialog graphs), SPMD over 8 NeuronCores.

Strategy
--------
The dialog graph is a causal 5-tap window (edges i -> i+o, o = 0..4, within
each 100-utterance dialog), and relation_type(i,j) = spk[i]*spk[j] with spk
derived from self-edges.  Every per-node linear map commutes with both the
window sum W(.) (row-mixing) and per-node diagonal scalings (row scaling), so
the whole network folds into 7-wide channels:

    out = W(g0) + f0
    g0  = RA + ic0.*W(A0) - ic0s.*W(spk.*A0) + ic1s.*W(spk.*A1) + cA*nv.*mask
    f0  = FSC + ic0.*W(B0) - ic0s.*W(spk.*B0) + ic1s.*W(spk.*B1) + cBc.*mask
    A{0,1} = x@(w_rel{0,1}@wA), B likewise with wB; RA = x@(w_root@wA),
    FSC = x@(w_root@wB + w_skip@w_clf); wA = w_gc_rel@w_clf, wB = w_gc_root@w_clf

Device work per column tile: one [1024 -> 56] bf16 matmul over x (the memory-
bound stream), one DVE multiply (ps * [spk|ones] -> Z rows 0:56), a 5-tap
window shift-tree over Z rows 0:42 batched two tiles per DVE op, a coefficient
multiply, and two tiny accumulating matmuls ([42->39] from the windowed V and
[32->39] from the plain Z rows + host-precomputed nv/mask constant rows).
The mask / neighbor-count channels are computed on host, so nothing else is
windowed on device.  Everything runs in bf16 (gate is 2e-2; this lands ~5e-3).

Layout: nodes sharded by dialog (no cross-core edges), 64 padded dialogs per
core; each dialog = 4 zero gap columns + 100 data columns so window sums never
leak across dialogs.  x ships transposed+tiled, one 1 MB DMA per column tile
(8 KiB contiguous per partition), all on the sync HWDGE ring (buffer-gated
issue); constants stream in parallel on the gpsimd SWDGE ring.
"""
import numpy as np
import ml_dtypes

BF16 = ml_dtypes.bfloat16

# ---------------------------------------------------------------- constants
B, L, FUT = 500, 100, 4
N = B * L
IN, HID, NCLS = 1024, 512, 7
NCORES = 8
GAP = 4
DLG = L + GAP            # 104 columns per dialog
DPC = 64                 # padded dialogs per core
COLS = DPC * DLG         # 6656 columns per core
NT = 13                  # column tiles
NTC = COLS // NT         # 512
KB = IN // 128           # 8 contraction blocks
M = 56                   # Wbig columns (psum partitions)
M2 = 39                  # S-matmul output columns
WIN = 42                 # windowed rows (0:28 spk-scaled, 28:42 plain)

D_COUNTS = [63, 63, 63, 63, 62, 62, 62, 62]
D_STARTS = np.concatenate([[0], np.cumsum(D_COUNTS)])[:-1]


def _data_cols():
    d = np.arange(DPC)[:, None]
    u = np.arange(L)[None, :]
    return d * DLG + GAP + u  # [DPC, L]


# ---------------------------------------------------------------- host prep
def _check_graph(edges, relation_type):
    i = np.arange(L)[:, None]
    off = np.arange(FUT + 1)[None, :]
    tl = i + off
    valid = tl < L
    sl = np.broadcast_to(i, tl.shape)[valid]
    tl = tl[valid]
    base = (np.arange(B) * L)[:, None]
    src = (base + sl[None, :]).reshape(-1)
    tgt = (base + tl[None, :]).reshape(-1)
    if edges.shape != (2, src.size) or not (
        np.array_equal(edges[0], src) and np.array_equal(edges[1], tgt)
    ):
        raise ValueError("edge structure does not match the DIAGCN pattern")
    sel = edges[0] == edges[1]
    spk = np.zeros(N, dtype=np.float64)
    spk[edges[0][sel]] = relation_type[sel]
    return spk


def _host_prep(x, edges, relation_type, w_rel, w_root, b_rgcn,
               w_gc_rel, w_gc_root, b_gc, w_skip, b_skip, w_clf, b_clf):
    x = np.asarray(x, dtype=np.float32)
    edges = np.asarray(edges)
    relation_type = np.asarray(relation_type)
    spk = _check_graph(edges, relation_type)

    tgt = edges[1]
    c1 = np.bincount(tgt[relation_type == 1], minlength=N).astype(np.float64)
    c0 = np.bincount(tgt[relation_type == 0], minlength=N).astype(np.float64)
    ic0 = 1.0 / np.maximum(c0, 1.0)
    ic1 = 1.0 / np.maximum(c1, 1.0)
    ic0s = ic0 * spk
    ic1s = ic1 * spk

    f8 = lambda a: np.asarray(a, dtype=np.float64)
    w_rel, w_root, w_gc_rel, w_gc_root, w_skip, w_clf = map(
        f8, (w_rel, w_root, w_gc_rel, w_gc_root, w_skip, w_clf))
    b_rgcn, b_gc, b_skip, b_clf = map(f8, (b_rgcn, b_gc, b_skip, b_clf))

    wA = w_gc_rel @ w_clf
    wB = w_gc_root @ w_clf
    # ps/Z rows: 0:7 A0(->A0S) 7:14 A1(->A1S) 14:21 B0(->B0S) 21:28 B1(->B1S)
    #            28:35 A0 plain, 35:42 B0 plain, 42:49 RA, 49:56 FSC
    Wbig = np.zeros((IN, M), dtype=np.float64)
    Wbig[:, 0:7] = w_rel[0] @ wA
    Wbig[:, 7:14] = w_rel[1] @ wA
    Wbig[:, 14:21] = w_rel[0] @ wB
    Wbig[:, 21:28] = w_rel[1] @ wB
    Wbig[:, 28:35] = w_rel[0] @ wA
    Wbig[:, 35:42] = w_rel[0] @ wB
    Wbig[:, 42:49] = w_root @ wA
    Wbig[:, 49:56] = w_root @ wB + w_skip @ w_clf
    # [128 partitions, KB, M]: partition p holds weight rows {k*128+p}
    Wbig = np.ascontiguousarray(
        Wbig.reshape(KB, 128, M).swapaxes(0, 1)).astype(BF16)

    cA = b_rgcn @ wA
    cBc = b_rgcn @ wB + (b_gc + b_skip) @ w_clf + b_clf
    # S_a: windowed+coefed V rows 0:42 -> ps2 (g0 at 0:7, f0 at 32:39)
    S_a = np.zeros((WIN, M2), dtype=np.float32)
    for i in range(7):
        S_a[0 + i, i] = 1.0          # -ic0s.*W(A0S)
        S_a[7 + i, i] = 1.0          # +ic1s.*W(A1S)
        S_a[28 + i, i] = 1.0         # ic0.*W(A0)
        S_a[14 + i, 32 + i] = 1.0    # -ic0s.*W(B0S)
        S_a[21 + i, 32 + i] = 1.0    # +ic1s.*W(B1S)
        S_a[35 + i, 32 + i] = 1.0    # ic0.*W(B0)
    # S_b: plain Z rows 32:64 -> ps2.  Z row 32+j maps to S_b row j:
    # rows 0:10 = A0[4:7],B0 plain (windowed copies, no direct contribution),
    # rows 10:17 = RA, 17:24 = FSC, 24 = nv*mask (cA), 25 = mask (cBc)
    S_b = np.zeros((32, M2), dtype=np.float32)
    for i in range(7):
        S_b[10 + i, i] = 1.0
        S_b[17 + i, 32 + i] = 1.0
    S_b[24, 0:7] = cA
    S_b[25, 32:39] = cBc
    S_a = S_a.astype(BF16)
    S_b = S_b.astype(BF16)

    dc = _data_cols()
    mask_col = np.zeros(COLS, dtype=np.float64)
    mask_col[dc.reshape(-1)] = 1.0
    nvm = np.convolve(mask_col, np.ones(FUT + 1))[:COLS] * mask_col
    zc = np.zeros((8, COLS), dtype=np.float32)  # -> Z rows 56:64
    zc[0] = nvm
    zc[1] = mask_col
    zc = zc.astype(BF16)

    in_maps = []
    unshard_info = []
    for c in range(NCORES):
        nd = D_COUNTS[c]
        g0 = D_STARTS[c]
        cols_real = dc[:nd].reshape(-1)
        nodes_real = g0 * L + np.arange(nd * L)

        xt = np.zeros((IN, COLS), dtype=np.float32)
        xt[:, cols_real] = x[nodes_real].T
        # swizzle: [NT][128 partitions][KB][NTC] so each column tile is one
        # DMA with 8 KiB contiguous per partition
        xts = np.ascontiguousarray(
            xt.reshape(KB, 128, NT, NTC).transpose(2, 1, 0, 3)).astype(BF16)

        def vec_to_cols(v):
            out = np.zeros(COLS, dtype=np.float32)
            out[cols_real] = v[nodes_real]
            return out

        spk_c = vec_to_cols(spk)
        ic0_c = vec_to_cols(ic0)
        ic0s_c = vec_to_cols(ic0s)
        ic1s_c = vec_to_cols(ic1s)

        spk32 = np.empty((32, COLS), dtype=np.float32)
        spk32[0:28] = spk_c
        spk32[28:32] = 1.0  # rows 28:32 of the spk|ones plane (rest is memset)
        coefrep = np.zeros((WIN, COLS), dtype=np.float32)
        coefrep[0:7] = -ic0s_c
        coefrep[7:14] = ic1s_c
        coefrep[14:21] = -ic0s_c
        coefrep[21:28] = ic1s_c
        coefrep[28:35] = ic0_c
        coefrep[35:42] = ic0_c

        in_maps.append(dict(
            xt=xts, wbig=Wbig, sa=S_a, sb=S_b, zc=zc,
            spk28=spk32.astype(BF16),
            coefrep=coefrep.astype(BF16),
        ))
        unshard_info.append((nodes_real, cols_real))
    return in_maps, unshard_info


# ---------------------------------------------------------------- bass kernel
_COMPILED = None


def _build():
    import concourse.bass as bass
    from concourse import bacc
    import concourse.mybir as mybir
    from concourse.tile import TileContext

    f32 = mybir.dt.float32
    bf16 = mybir.dt.bfloat16
    ADD = mybir.AluOpType.add
    MUL = mybir.AluOpType.mult

    nc = bacc.Bacc("TRN2", target_bir_lowering=False, debug=False,
                   num_devices=NCORES)
    xt_d = nc.dram_tensor("xt", [NT, 128, KB, NTC], bf16, kind="ExternalInput")
    wbig_d = nc.dram_tensor("wbig", [128, KB, M], bf16, kind="ExternalInput")
    sa_d = nc.dram_tensor("sa", [WIN, M2], bf16, kind="ExternalInput")
    sb_d = nc.dram_tensor("sb", [32, M2], bf16, kind="ExternalInput")
    zc_d = nc.dram_tensor("zc", [8, COLS], bf16, kind="ExternalInput")
    spk_d = nc.dram_tensor("spk28", [32, COLS], bf16, kind="ExternalInput")
    coef_d = nc.dram_tensor("coefrep", [WIN, COLS], bf16, kind="ExternalInput")
    y_d = nc.dram_tensor("y", [NCLS, COLS], bf16, kind="ExternalOutput")

    with TileContext(nc) as tc:
        with (
            tc.tile_pool(name="const", bufs=1) as cpool,
            tc.tile_pool(name="xin", bufs=4) as xpool,
            tc.tile_pool(name="wrk", bufs=3) as wpool,
            tc.tile_pool(name="g2", bufs=1) as gpool,
            tc.tile_pool(name="psum", bufs=4, space="PSUM") as ppool,
            tc.tile_pool(name="psum2", bufs=4, space="PSUM") as p2pool,
        ):
            wsb = cpool.tile([128, KB, M], bf16)
            nc.sync.dma_start(wsb[:], wbig_d[:])
            sasb = cpool.tile([WIN, M2], bf16)
            nc.sync.dma_start(sasb[:], sa_d[:])
            # S_b parks at partitions 32:64 — matmul requires lhsT and rhs to
            # share the same base partition, and its rhs is tZ[32:64]
            sbt = cpool.tile([64, M2], bf16)
            nc.sync.dma_start(sbt[32:64], sb_d[:])
            sbsb = sbt[32:64]

            # persistent [*, COLS] planes (SBUF free-dim bytes are charged per
            # partition regardless of row count).  Engine-op APs must start at
            # partition 0/32/64/96 and a non-zero base spans at most 32
            # partitions, so the >32-row tensors each sit at base 0.
            tZ = cpool.tile([64, COLS], bf16)    # Z: 0:56 per-tile, 56:64 consts
            tSP = cpool.tile([M, COLS], bf16)    # spk|ones
            tT1 = cpool.tile([WIN, COLS], bf16)  # window stage 1 (persists)
            tV = cpool.tile([WIN, COLS], bf16)   # windowed * coef
            tCF = cpool.tile([WIN, COLS], bf16)  # coefficients
            tGP = cpool.tile([128, COLS // 4], bf16)  # packed g0: group g rows 32g..32g+6
            tOP = cpool.tile([128, COLS // 4], bf16)  # packed out, same layout

            # concurrent DMAs share HBM bandwidth round-robin, so the first
            # tiles must be queued ahead of the (large) constant planes or the
            # first matmul starts late.  Everything rides the sync HWDGE ring.
            # Tiles 0/1 are split into k-halves so the first matmuls can start
            # after 512 KB (subtile deps track the halves separately).
            xts = {}
            for t in (0, 1):
                xts[t] = xpool.tile([128, KB, NTC], bf16, tag="xt", name="xt_t")
                nc.sync.dma_start(xts[t][:, 0:KB // 2, :], xt_d[t, :, 0:KB // 2, :])
                nc.sync.dma_start(xts[t][:, KB // 2:, :], xt_d[t, :, KB // 2:, :])
            nc.sync.dma_start(tZ[56:64], zc_d[:])
            nc.sync.dma_start(tSP[0:32], spk_d[:])
            xts[2] = xpool.tile([128, KB, NTC], bf16, tag="xt", name="xt_t")
            nc.sync.dma_start(xts[2][:], xt_d[2])
            nc.sync.dma_start(tCF[:], coef_d[:])
            nc.vector.memset(tSP[32:M], 1.0)
            nc.vector.memset(tGP[:], 0.0)
            nc.vector.memset(tOP[:], 0.0)
            # dummy copy so the one-time ACT table load happens at startup,
            # not in front of the first real PSUM->SBUF copy
            nc.scalar.copy(tT1[0:1, 0:8], sbt[32:33, 0:8])

            GRP = COLS // 4  # 1664, a whole number of dialogs

            def finalize_pair(tlo, thi):
                # second S-matmul + PSUM->packed copies; called one pair late
                # so the tensor queue never stalls waiting on the DVE chain
                for t in range(tlo, thi):
                    c0, c1 = t * NTC, (t + 1) * NTC
                    ps2 = ps2s.pop(t)
                    nc.tensor.matmul(ps2[:], sasb[:], tV[:, c0:c1],
                                     start=False, stop=True)
                    for (glo, ghi) in [(c0, min(c1, (c0 // GRP + 1) * GRP)),
                                       ((c0 // GRP + 1) * GRP, c1)]:
                        if glo >= ghi:
                            continue
                        g = glo // GRP
                        nc.scalar.copy(
                            tGP[32 * g:32 * g + NCLS, glo - g * GRP:ghi - g * GRP],
                            ps2[0:NCLS, glo - c0:ghi - c0])
                        nc.scalar.copy(
                            tOP[32 * g:32 * g + NCLS, glo - g * GRP:ghi - g * GRP],
                            ps2[32:32 + NCLS, glo - c0:ghi - c0])

            # win2: 5-tap window of packed g0, all 4 groups per op (rows
            # 32g..32g+6).  Chunked so most of it overlaps the main loop.
            NR = 96 + NCLS
            gs1 = gpool.tile([NR, GRP], bf16, tag="gs1")
            gp = tGP[0:NR]

            def win2_chunk(b0, b1):
                gs2 = gpool.tile([NR, b1 - b0], bf16, tag="gs2")
                gwt = gpool.tile([NR, b1 - b0], bf16, tag="gwt")
                if b0 == 0:
                    nc.vector.tensor_copy(gs1[:, 0:1], gp[:, 0:1])
                    nc.vector.tensor_tensor(gs1[:, 1:b1], gp[:, 1:b1], gp[:, 0:b1 - 1], ADD)
                    nc.vector.tensor_copy(gs2[:, 0:2], gs1[:, 0:2])
                    nc.vector.tensor_tensor(gs2[:, 2:], gs1[:, 2:b1], gs1[:, 0:b1 - 2], ADD)
                    nc.vector.tensor_copy(gwt[:, 0:4], gs2[:, 0:4])
                    nc.vector.tensor_tensor(gwt[:, 4:], gs2[:, 4:], gp[:, 0:b1 - 4], ADD)
                else:
                    nc.vector.tensor_tensor(gs1[:, b0:b1], gp[:, b0:b1], gp[:, b0 - 1:b1 - 1], ADD)
                    nc.vector.tensor_tensor(gs2[:], gs1[:, b0:b1], gs1[:, b0 - 2:b1 - 2], ADD)
                    nc.vector.tensor_tensor(gwt[:], gs2[:], gp[:, b0 - 4:b1 - 4], ADD)
                nc.vector.tensor_tensor(tOP[0:NR, b0:b1], tOP[0:NR, b0:b1], gwt[:], ADD)

            B1 = 12 * NTC - 3 * GRP  # group-3 columns complete after tile 11

            ps2s = {}
            pairs = [(2 * p, min(2 * p + 2, NT)) for p in range((NT + 1) // 2)]
            for (tlo, thi) in pairs:
                for t in range(tlo, thi):
                    if t not in xts:
                        xts[t] = xpool.tile([128, KB, NTC], bf16, tag="xt", name="xt_t")
                        nc.sync.dma_start(xts[t][:], xt_d[t])
                    xt_t = xts.pop(t)
                    ps = ppool.tile([M, NTC], f32)
                    for k in range(KB):
                        nc.tensor.matmul(
                            ps[:], wsb[:, k, :], xt_t[:, k, :],
                            start=(k == 0), stop=(k == KB - 1))
                    c0, c1 = t * NTC, (t + 1) * NTC
                    # rows 0:28 spk-scaled, 28:56 plain (spkx rows are 1.0)
                    nc.vector.tensor_tensor(tZ[0:M, c0:c1], ps[:], tSP[:, c0:c1], MUL)
                    # first S-matmul half: plain rows + host consts, needs only Z
                    ps2s[t] = p2pool.tile([M2, NTC], f32, tag="ps2", name="ps2")
                    nc.tensor.matmul(ps2s[t][:], sbsb, tZ[32:64, c0:c1],
                                     start=True, stop=False)

                # 5-tap causal window over the pair's columns as a shift tree:
                #   t1 = z + sh1(z); t2 = t1 + sh2(t1); wt = t2 + sh4(z)
                C0, C1 = tlo * NTC, thi * NTC
                W = C1 - C0
                Z = tZ[0:WIN]
                T1 = tT1
                T2 = wpool.tile([WIN, W], bf16, tag="T2")
                WT = wpool.tile([WIN, W], bf16, tag="WT")
                if tlo == 0:
                    nc.vector.tensor_copy(T1[:, 0:1], Z[:, 0:1])
                    nc.vector.tensor_tensor(T1[:, 1:C1], Z[:, 1:C1], Z[:, 0:C1 - 1], ADD)
                    nc.vector.tensor_copy(T2[:, 0:2], T1[:, 0:2])
                    nc.vector.tensor_tensor(T2[:, 2:], T1[:, 2:C1], T1[:, 0:C1 - 2], ADD)
                    nc.vector.tensor_copy(WT[:, 0:4], T2[:, 0:4])
                    nc.vector.tensor_tensor(WT[:, 4:], T2[:, 4:], Z[:, 0:C1 - 4], ADD)
                else:
                    nc.vector.tensor_tensor(T1[:, C0:C1], Z[:, C0:C1], Z[:, C0 - 1:C1 - 1], ADD)
                    nc.vector.tensor_tensor(T2[:], T1[:, C0:C1], T1[:, C0 - 2:C1 - 2], ADD)
                    nc.vector.tensor_tensor(WT[:], T2[:], Z[:, C0 - 4:C1 - 4], ADD)
                nc.vector.tensor_tensor(tV[:, C0:C1], WT[:], tCF[:, C0:C1], MUL)

                # lag finalize by one pair (immediate for the last two pairs:
                # pair 5 so win2 chunk 1 can start early, pair 6 as the tail)
                prev = pairs.index((tlo, thi)) - 1
                if prev >= 0 and ps2s.get(pairs[prev][0]) is not None:
                    finalize_pair(*pairs[prev])
                if (tlo, thi) == pairs[-2]:
                    finalize_pair(tlo, thi)
                elif (tlo, thi) == pairs[-1]:
                    win2_chunk(0, B1)
                    finalize_pair(tlo, thi)

            win2_chunk(B1, GRP)
            for g in range(4):
                eng = nc.sync if g % 2 == 0 else nc.scalar
                eng.dma_start(y_d[:, g * GRP:(g + 1) * GRP],
                              tOP[32 * g:32 * g + NCLS, :])
    nc.compile()
    return nc


def _get_compiled():
    global _COMPILED
    if _COMPILED is None:
        _COMPILED = _build()
    return _COMPILED


def _run(in_maps, trace=False):
    from concourse.bass_utils import run_bass_kernel_spmd
    nc = _get_compiled()
    return run_bass_kernel_spmd(nc, in_maps, list(range(NCORES)), trace=trace)


def kernel(**inputs) -> np.ndarray:
    in_maps, unshard_info = _host_prep(**inputs)
    res = _run(in_maps)
    out = np.zeros((N, NCLS), dtype=np.float32)
    for c in range(NCORES):
        nodes_real, cols_real = unshard_info[c]
        out[nodes_real] = res.results[c]["y"][:, cols_real].T.astype(np.float32)
    return out


# revision 15
# speedup vs baseline: 1.1531x; 1.0862x over previous
"""Trainium2 Bass kernel for nn_DIAGCN (RGCN + GraphConv + classifier over
block-diagonal dialog graphs), SPMD over 8 NeuronCores.

Strategy
--------
The dialog graph is a causal 5-tap window (edges i -> i+o, o = 0..4, within
each 100-utterance dialog), and relation_type(i,j) = spk[i]*spk[j] with spk
derived from self-edges.  Every per-node linear map commutes with both the
window sum W(.) (row-mixing) and per-node diagonal scalings (row scaling), so
the whole network folds into 7-wide channels:

    out = W(g0) + f0
    g0  = RA + ic0.*W(A0) - ic0s.*W(spk.*A0) + ic1s.*W(spk.*A1) + cA*nv.*mask
    f0  = FSC + ic0.*W(B0) - ic0s.*W(spk.*B0) + ic1s.*W(spk.*B1) + cBc.*mask
    A{0,1} = x@(w_rel{0,1}@wA), B likewise with wB; RA = x@(w_root@wA),
    FSC = x@(w_root@wB + w_skip@w_clf); wA = w_gc_rel@w_clf, wB = w_gc_root@w_clf

Device work per column tile: one [1024 -> 56] bf16 matmul over x (the memory-
bound stream), one DVE multiply (ps * [spk|ones] -> Z rows 0:56), a 5-tap
window shift-tree over Z rows 0:42 batched two tiles per DVE op, a coefficient
multiply, and two tiny accumulating matmuls ([42->39] from the windowed V and
[32->39] from the plain Z rows + host-precomputed nv/mask constant rows).
The mask / neighbor-count channels are computed on host, so nothing else is
windowed on device.  Everything runs in bf16 (gate is 2e-2; this lands ~5e-3).

Layout: nodes sharded by dialog (no cross-core edges), 64 padded dialogs per
core; each dialog = 4 zero gap columns + 100 data columns so window sums never
leak across dialogs.  x ships transposed+tiled, one 1 MB DMA per column tile
(8 KiB contiguous per partition), all on the sync HWDGE ring (buffer-gated
issue); constants stream in parallel on the gpsimd SWDGE ring.
"""
import numpy as np
import ml_dtypes

BF16 = ml_dtypes.bfloat16

# ---------------------------------------------------------------- constants
B, L, FUT = 500, 100, 4
N = B * L
IN, HID, NCLS = 1024, 512, 7
NCORES = 8
GAP = 4
DLG = L + GAP            # 104 columns per dialog
DPC = 64                 # padded dialogs per core
COLS = DPC * DLG         # 6656 columns per core
NT = 13                  # column tiles
NTC = COLS // NT         # 512
KB = IN // 128           # 8 contraction blocks
M = 56                   # Wbig columns (psum partitions)
M2 = 39                  # S-matmul output columns
WIN = 42                 # windowed rows (0:28 spk-scaled, 28:42 plain)

D_COUNTS = [63, 63, 63, 63, 62, 62, 62, 62]
D_STARTS = np.concatenate([[0], np.cumsum(D_COUNTS)])[:-1]


def _data_cols():
    d = np.arange(DPC)[:, None]
    u = np.arange(L)[None, :]
    return d * DLG + GAP + u  # [DPC, L]


# ---------------------------------------------------------------- host prep
def _check_graph(edges, relation_type):
    i = np.arange(L)[:, None]
    off = np.arange(FUT + 1)[None, :]
    tl = i + off
    valid = tl < L
    sl = np.broadcast_to(i, tl.shape)[valid]
    tl = tl[valid]
    base = (np.arange(B) * L)[:, None]
    src = (base + sl[None, :]).reshape(-1)
    tgt = (base + tl[None, :]).reshape(-1)
    if edges.shape != (2, src.size) or not (
        np.array_equal(edges[0], src) and np.array_equal(edges[1], tgt)
    ):
        raise ValueError("edge structure does not match the DIAGCN pattern")
    sel = edges[0] == edges[1]
    spk = np.zeros(N, dtype=np.float64)
    spk[edges[0][sel]] = relation_type[sel]
    return spk


def _host_prep(x, edges, relation_type, w_rel, w_root, b_rgcn,
               w_gc_rel, w_gc_root, b_gc, w_skip, b_skip, w_clf, b_clf):
    x = np.asarray(x, dtype=np.float32)
    edges = np.asarray(edges)
    relation_type = np.asarray(relation_type)
    spk = _check_graph(edges, relation_type)

    tgt = edges[1]
    c1 = np.bincount(tgt[relation_type == 1], minlength=N).astype(np.float64)
    c0 = np.bincount(tgt[relation_type == 0], minlength=N).astype(np.float64)
    ic0 = 1.0 / np.maximum(c0, 1.0)
    ic1 = 1.0 / np.maximum(c1, 1.0)
    ic0s = ic0 * spk
    ic1s = ic1 * spk

    f8 = lambda a: np.asarray(a, dtype=np.float64)
    w_rel, w_root, w_gc_rel, w_gc_root, w_skip, w_clf = map(
        f8, (w_rel, w_root, w_gc_rel, w_gc_root, w_skip, w_clf))
    b_rgcn, b_gc, b_skip, b_clf = map(f8, (b_rgcn, b_gc, b_skip, b_clf))

    wA = w_gc_rel @ w_clf
    wB = w_gc_root @ w_clf
    # ps/Z rows: 0:7 A0(->A0S) 7:14 A1(->A1S) 14:21 B0(->B0S) 21:28 B1(->B1S)
    #            28:35 A0 plain, 35:42 B0 plain, 42:49 RA, 49:56 FSC
    Wbig = np.zeros((IN, M), dtype=np.float64)
    Wbig[:, 0:7] = w_rel[0] @ wA
    Wbig[:, 7:14] = w_rel[1] @ wA
    Wbig[:, 14:21] = w_rel[0] @ wB
    Wbig[:, 21:28] = w_rel[1] @ wB
    Wbig[:, 28:35] = w_rel[0] @ wA
    Wbig[:, 35:42] = w_rel[0] @ wB
    Wbig[:, 42:49] = w_root @ wA
    Wbig[:, 49:56] = w_root @ wB + w_skip @ w_clf
    # [128 partitions, KB, M]: partition p holds weight rows {k*128+p}
    Wbig = np.ascontiguousarray(
        Wbig.reshape(KB, 128, M).swapaxes(0, 1)).astype(BF16)

    cA = b_rgcn @ wA
    cBc = b_rgcn @ wB + (b_gc + b_skip) @ w_clf + b_clf
    # S_a: windowed+coefed V rows 0:42 -> ps2 (g0 at 0:7, f0 at 32:39)
    S_a = np.zeros((WIN, M2), dtype=np.float32)
    for i in range(7):
        S_a[0 + i, i] = 1.0          # -ic0s.*W(A0S)
        S_a[7 + i, i] = 1.0          # +ic1s.*W(A1S)
        S_a[28 + i, i] = 1.0         # ic0.*W(A0)
        S_a[14 + i, 32 + i] = 1.0    # -ic0s.*W(B0S)
        S_a[21 + i, 32 + i] = 1.0    # +ic1s.*W(B1S)
        S_a[35 + i, 32 + i] = 1.0    # ic0.*W(B0)
    # S_b: plain Z rows 32:64 -> ps2.  Z row 32+j maps to S_b row j:
    # rows 0:10 = A0[4:7],B0 plain (windowed copies, no direct contribution),
    # rows 10:17 = RA, 17:24 = FSC, 24 = nv*mask (cA), 25 = mask (cBc)
    S_b = np.zeros((32, M2), dtype=np.float32)
    for i in range(7):
        S_b[10 + i, i] = 1.0
        S_b[17 + i, 32 + i] = 1.0
    S_b[24, 0:7] = cA
    S_b[25, 32:39] = cBc
    S_a = S_a.astype(BF16)
    S_b = S_b.astype(BF16)

    dc = _data_cols()
    mask_col = np.zeros(COLS, dtype=np.float64)
    mask_col[dc.reshape(-1)] = 1.0
    nvm = np.convolve(mask_col, np.ones(FUT + 1))[:COLS] * mask_col
    zc = np.zeros((8, COLS), dtype=np.float32)  # -> Z rows 56:64
    zc[0] = nvm
    zc[1] = mask_col
    zc = zc.astype(BF16)

    in_maps = []
    unshard_info = []
    for c in range(NCORES):
        nd = D_COUNTS[c]
        g0 = D_STARTS[c]
        cols_real = dc[:nd].reshape(-1)
        nodes_real = g0 * L + np.arange(nd * L)

        xt = np.zeros((IN, COLS), dtype=np.float32)
        xt[:, cols_real] = x[nodes_real].T
        # swizzle: [NT][128 partitions][KB][NTC] so each column tile is one
        # DMA with 8 KiB contiguous per partition
        xts = np.ascontiguousarray(
            xt.reshape(KB, 128, NT, NTC).transpose(2, 1, 0, 3)).astype(BF16)

        def vec_to_cols(v):
            out = np.zeros(COLS, dtype=np.float32)
            out[cols_real] = v[nodes_real]
            return out

        spk_c = vec_to_cols(spk)
        ic0_c = vec_to_cols(ic0)
        ic0s_c = vec_to_cols(ic0s)
        ic1s_c = vec_to_cols(ic1s)

        spk32 = np.empty((32, COLS), dtype=np.float32)
        spk32[0:28] = spk_c
        spk32[28:32] = 1.0  # rows 28:32 of the spk|ones plane (rest is memset)
        coefrep = np.zeros((WIN, COLS), dtype=np.float32)
        coefrep[0:7] = -ic0s_c
        coefrep[7:14] = ic1s_c
        coefrep[14:21] = -ic0s_c
        coefrep[21:28] = ic1s_c
        coefrep[28:35] = ic0_c
        coefrep[35:42] = ic0_c

        in_maps.append(dict(
            xt=xts, wbig=Wbig, sa=S_a, sb=S_b, zc=zc,
            spk28=spk32.astype(BF16),
            coefrep=coefrep.astype(BF16),
        ))
        unshard_info.append((nodes_real, cols_real))
    return in_maps, unshard_info


# ---------------------------------------------------------------- bass kernel
_COMPILED = None


def _build():
    import concourse.bass as bass
    from concourse import bacc
    import concourse.mybir as mybir
    from concourse.tile import TileContext

    f32 = mybir.dt.float32
    bf16 = mybir.dt.bfloat16
    ADD = mybir.AluOpType.add
    MUL = mybir.AluOpType.mult

    nc = bacc.Bacc("TRN2", target_bir_lowering=False, debug=False,
                   num_devices=NCORES)
    xt_d = nc.dram_tensor("xt", [NT, 128, KB, NTC], bf16, kind="ExternalInput")
    wbig_d = nc.dram_tensor("wbig", [128, KB, M], bf16, kind="ExternalInput")
    sa_d = nc.dram_tensor("sa", [WIN, M2], bf16, kind="ExternalInput")
    sb_d = nc.dram_tensor("sb", [32, M2], bf16, kind="ExternalInput")
    zc_d = nc.dram_tensor("zc", [8, COLS], bf16, kind="ExternalInput")
    spk_d = nc.dram_tensor("spk28", [32, COLS], bf16, kind="ExternalInput")
    coef_d = nc.dram_tensor("coefrep", [WIN, COLS], bf16, kind="ExternalInput")
    y_d = nc.dram_tensor("y", [NCLS, COLS], bf16, kind="ExternalOutput")

    with TileContext(nc) as tc:
        with (
            tc.tile_pool(name="const", bufs=1) as cpool,
            tc.tile_pool(name="xin", bufs=8) as xpool,
            tc.tile_pool(name="wrk", bufs=3) as wpool,
            tc.tile_pool(name="g2", bufs=1) as gpool,
            tc.tile_pool(name="psum", bufs=4, space="PSUM") as ppool,
            tc.tile_pool(name="psum2", bufs=4, space="PSUM") as p2pool,
        ):
            wsb = cpool.tile([128, KB, M], bf16)
            nc.sync.dma_start(wsb[:], wbig_d[:])
            sasb = cpool.tile([WIN, M2], bf16)
            nc.sync.dma_start(sasb[:], sa_d[:])
            # S_b parks at partitions 32:64 — matmul requires lhsT and rhs to
            # share the same base partition, and its rhs is tZ[32:64]
            sbt = cpool.tile([64, M2], bf16)
            nc.sync.dma_start(sbt[32:64], sb_d[:])
            sbsb = sbt[32:64]

            # persistent [*, COLS] planes (SBUF free-dim bytes are charged per
            # partition regardless of row count).  Engine-op APs must start at
            # partition 0/32/64/96 and a non-zero base spans at most 32
            # partitions, so the >32-row tensors each sit at base 0.
            tZ = cpool.tile([64, COLS], bf16)    # Z: 0:56 per-tile, 56:64 consts
            tSP = cpool.tile([M, COLS], bf16)    # spk|ones
            tT1 = cpool.tile([WIN, COLS], bf16)  # window stage 1 (persists)
            tV = cpool.tile([WIN, COLS], bf16)   # windowed * coef
            tCF = cpool.tile([WIN, COLS], bf16)  # coefficients
            tGP = cpool.tile([128, COLS // 4], bf16)  # packed g0: group g rows 32g..32g+6
            tOP = cpool.tile([128, COLS // 4], bf16)  # packed out, same layout

            # concurrent DMAs share HBM bandwidth round-robin, so the first
            # tiles must hit the engines before the (large) constant planes or
            # the first matmul starts late.  xt tiles ride the sync HWDGE ring
            # (deep bufs=8 queue keeps the SDMA engines pipelined); constants
            # ride the gpsimd SWDGE ring, sequenced BEHIND the ~6us tSP memset
            # so their descriptors join the melee only after the first tiles.
            # Tiles 0/1 are split into k-halves so the first matmuls can start
            # after 512 KB (subtile deps track the halves separately).
            xts = {}
            for t in (0, 1):
                xts[t] = xpool.tile([128, KB, NTC], bf16, tag="xt", name="xt_t")
                nc.sync.dma_start(xts[t][:, 0:KB // 2, :], xt_d[t, :, 0:KB // 2, :])
                nc.sync.dma_start(xts[t][:, KB // 2:, :], xt_d[t, :, KB // 2:, :])
            nc.gpsimd.memset(tSP[32:M], 1.0)
            nc.gpsimd.dma_start(tZ[56:64], zc_d[:])
            nc.gpsimd.dma_start(tSP[0:32], spk_d[:])
            nc.gpsimd.dma_start(tCF[:], coef_d[:])
            nc.gpsimd.memset(tGP[:], 0.0)
            nc.gpsimd.memset(tOP[:], 0.0)
            # dummy copy so the one-time ACT table load happens at startup,
            # not in front of the first real PSUM->SBUF copy
            nc.scalar.copy(tT1[0:1, 0:8], sbt[32:33, 0:8])

            GRP = COLS // 4  # 1664, a whole number of dialogs

            def finalize_pair(tlo, thi):
                # second S-matmul + PSUM->packed copies; called one pair late
                # so the tensor queue never stalls waiting on the DVE chain
                for t in range(tlo, thi):
                    c0, c1 = t * NTC, (t + 1) * NTC
                    ps2 = ps2s.pop(t)
                    nc.tensor.matmul(ps2[:], sasb[:], tV[:, c0:c1],
                                     start=False, stop=True)
                    for (glo, ghi) in [(c0, min(c1, (c0 // GRP + 1) * GRP)),
                                       ((c0 // GRP + 1) * GRP, c1)]:
                        if glo >= ghi:
                            continue
                        g = glo // GRP
                        nc.scalar.copy(
                            tGP[32 * g:32 * g + NCLS, glo - g * GRP:ghi - g * GRP],
                            ps2[0:NCLS, glo - c0:ghi - c0])
                        nc.scalar.copy(
                            tOP[32 * g:32 * g + NCLS, glo - g * GRP:ghi - g * GRP],
                            ps2[32:32 + NCLS, glo - c0:ghi - c0])

            # win2: 5-tap window of packed g0, all 4 groups per op (rows
            # 32g..32g+6).  Chunked so most of it overlaps the main loop.
            NR = 96 + NCLS
            gs1 = gpool.tile([NR, GRP], bf16, tag="gs1")
            gp = tGP[0:NR]

            def win2_chunk(b0, b1):
                gs2 = gpool.tile([NR, b1 - b0], bf16, tag="gs2")
                gwt = gpool.tile([NR, b1 - b0], bf16, tag="gwt")
                if b0 == 0:
                    nc.vector.tensor_copy(gs1[:, 0:1], gp[:, 0:1])
                    nc.vector.tensor_tensor(gs1[:, 1:b1], gp[:, 1:b1], gp[:, 0:b1 - 1], ADD)
                    nc.vector.tensor_copy(gs2[:, 0:2], gs1[:, 0:2])
                    nc.vector.tensor_tensor(gs2[:, 2:], gs1[:, 2:b1], gs1[:, 0:b1 - 2], ADD)
                    nc.vector.tensor_copy(gwt[:, 0:4], gs2[:, 0:4])
                    nc.vector.tensor_tensor(gwt[:, 4:], gs2[:, 4:], gp[:, 0:b1 - 4], ADD)
                else:
                    nc.vector.tensor_tensor(gs1[:, b0:b1], gp[:, b0:b1], gp[:, b0 - 1:b1 - 1], ADD)
                    nc.vector.tensor_tensor(gs2[:], gs1[:, b0:b1], gs1[:, b0 - 2:b1 - 2], ADD)
                    nc.vector.tensor_tensor(gwt[:], gs2[:], gp[:, b0 - 4:b1 - 4], ADD)
                nc.vector.tensor_tensor(tOP[0:NR, b0:b1], tOP[0:NR, b0:b1], gwt[:], ADD)

            B1 = 12 * NTC - 3 * GRP  # group-3 columns complete after tile 11

            ps2s = {}
            pairs = [(2 * p, min(2 * p + 2, NT)) for p in range((NT + 1) // 2)]
            for (tlo, thi) in pairs:
                for t in range(tlo, thi):
                    if t not in xts:
                        xts[t] = xpool.tile([128, KB, NTC], bf16, tag="xt", name="xt_t")
                        nc.sync.dma_start(xts[t][:], xt_d[t])
                    xt_t = xts.pop(t)
                    ps = ppool.tile([M, NTC], f32)
                    for k in range(KB):
                        nc.tensor.matmul(
                            ps[:], wsb[:, k, :], xt_t[:, k, :],
                            start=(k == 0), stop=(k == KB - 1))
                    c0, c1 = t * NTC, (t + 1) * NTC
                    # rows 0:28 spk-scaled, 28:56 plain (spkx rows are 1.0)
                    nc.vector.tensor_tensor(tZ[0:M, c0:c1], ps[:], tSP[:, c0:c1], MUL)
                    # first S-matmul half: plain rows + host consts, needs only Z
                    ps2s[t] = p2pool.tile([M2, NTC], f32, tag="ps2", name="ps2")
                    nc.tensor.matmul(ps2s[t][:], sbsb, tZ[32:64, c0:c1],
                                     start=True, stop=False)

                # 5-tap causal window over the pair's columns as a shift tree:
                #   t1 = z + sh1(z); t2 = t1 + sh2(t1); wt = t2 + sh4(z)
                C0, C1 = tlo * NTC, thi * NTC
                W = C1 - C0
                Z = tZ[0:WIN]
                T1 = tT1
                T2 = wpool.tile([WIN, W], bf16, tag="T2")
                WT = wpool.tile([WIN, W], bf16, tag="WT")
                if tlo == 0:
                    nc.vector.tensor_copy(T1[:, 0:1], Z[:, 0:1])
                    nc.vector.tensor_tensor(T1[:, 1:C1], Z[:, 1:C1], Z[:, 0:C1 - 1], ADD)
                    nc.vector.tensor_copy(T2[:, 0:2], T1[:, 0:2])
                    nc.vector.tensor_tensor(T2[:, 2:], T1[:, 2:C1], T1[:, 0:C1 - 2], ADD)
                    nc.vector.tensor_copy(WT[:, 0:4], T2[:, 0:4])
                    nc.vector.tensor_tensor(WT[:, 4:], T2[:, 4:], Z[:, 0:C1 - 4], ADD)
                else:
                    nc.vector.tensor_tensor(T1[:, C0:C1], Z[:, C0:C1], Z[:, C0 - 1:C1 - 1], ADD)
                    nc.vector.tensor_tensor(T2[:], T1[:, C0:C1], T1[:, C0 - 2:C1 - 2], ADD)
                    nc.vector.tensor_tensor(WT[:], T2[:], Z[:, C0 - 4:C1 - 4], ADD)
                nc.vector.tensor_tensor(tV[:, C0:C1], WT[:], tCF[:, C0:C1], MUL)

                # lag finalize by one pair (immediate for the last two pairs:
                # pair 5 so win2 chunk 1 can start early, pair 6 as the tail)
                prev = pairs.index((tlo, thi)) - 1
                if prev >= 0 and ps2s.get(pairs[prev][0]) is not None:
                    finalize_pair(*pairs[prev])
                if (tlo, thi) == pairs[-2]:
                    finalize_pair(tlo, thi)
                elif (tlo, thi) == pairs[-1]:
                    win2_chunk(0, B1)
                    finalize_pair(tlo, thi)

            win2_chunk(B1, GRP)
            for g in range(4):
                eng = nc.sync if g % 2 == 0 else nc.scalar
                eng.dma_start(y_d[:, g * GRP:(g + 1) * GRP],
                              tOP[32 * g:32 * g + NCLS, :])
    nc.compile()
    return nc


def _get_compiled():
    global _COMPILED
    if _COMPILED is None:
        _COMPILED = _build()
    return _COMPILED


def _run(in_maps, trace=False):
    from concourse.bass_utils import run_bass_kernel_spmd
    nc = _get_compiled()
    return run_bass_kernel_spmd(nc, in_maps, list(range(NCORES)), trace=trace)


def kernel(**inputs) -> np.ndarray:
    in_maps, unshard_info = _host_prep(**inputs)
    res = _run(in_maps)
    out = np.zeros((N, NCLS), dtype=np.float32)
    for c in range(NCORES):
        nodes_real, cols_real = unshard_info[c]
        out[nodes_real] = res.results[c]["y"][:, cols_real].T.astype(np.float32)
    return out
